# revision 1
# baseline (speedup 1.0000x reference)
"""Attention2d Trainium2 kernel.

Reference computation (per sample b):
  K = Wk @ x + bk;  Q = Wq @ x + bq;  V = Wv @ x + bv     (x: [128, 1024])
  per head h (32 channels):  att[k,q] = scale * K_h[:,k].Q_h[:,q] + rel_h[k,q]
  P = softmax_k(att);  out_h = V_h @ P;  y = Wu @ out + bu

Kernel strategy (8 NeuronCores, data-parallel over batch, 2 samples/core):
  - host: transpose weights (lhsT layouts), fold `scale` into Wq, gather
    rel = pos_enc[:, idx] -> bf16, fold bv/bu into one final bias (softmax
    column-sums are 1, so V-bias passes through attention unchanged), drop
    bk (constant-in-k shift, softmax-invariant).
  - att computed in [k_part, q_free] layout; rel added by an identity
    matmul accumulating into the same PSUM bank; exp on ScalarE.
  - softmax denominator D[q] via an appended ones-column in the V^T
    stationary operand (row 32 of the 2nd matmul output), division applied
    after the 2nd matmul via a selector-matmul partition-broadcast of 1/D.
  - all matmuls run as float32r (full-rate fp32 streaming on PE).
"""

import os
import sys
import types

sys.path.insert(0, "/opt/trn_rl_repo")

import numpy as np
import ml_dtypes

import concourse.bass as bass
import concourse.tile as tile
from concourse import bacc, mybir
from concourse import bass_utils
from concourse.bass import ds, ts

F32 = mybir.dt.float32
F32R = mybir.dt.float32r
F16 = mybir.dt.float16
BF16 = mybir.dt.bfloat16
AF = mybir.ActivationFunctionType

B, E, H, NY, NX = 16, 128, 4, 32, 32
N = NY * NX          # 1024
HC = E // H          # 32
NCORES = 8
BPC = B // NCORES    # 2 samples per core
NT = N // 128        # 8 k-tiles
SCALE = HC ** -0.5

LAST_RESULT = None   # BassKernelResults of the most recent run (for test.py)

_CACHE = {}


def _patch_ldw_opt():
    """Enable walrus LDWEIGHTS elision (redundant identity reloads)."""
    if _CACHE.get("ldw_patched"):
        return
    import concourse.bass_utils as _bu
    orig = _bu.run_command

    def patched(argv, **kw):
        argv = [a.replace("--enable-ldw-opt=false", "--enable-ldw-opt=true")
                if isinstance(a, str) else a for a in argv]
        return orig(argv, **kw)

    _bu.run_command = patched
    _CACHE["ldw_patched"] = True


def _ensure_ntff_hook():
    """Register the axon NTFF profile hook that trn_boot couldn't install
    (the image lacks antenv.axon_hooks). Only needed when tracing."""
    if "antenv.axon_hooks" in sys.modules:
        return
    mod = types.ModuleType("antenv.axon_hooks")
    holder = [None]
    mod.set_axon_ntff_profile_hook = lambda h: holder.__setitem__(0, h)
    mod.get_axon_ntff_profile_hook = lambda: holder[0]
    sys.modules["antenv.axon_hooks"] = mod
    try:
        from trn_agent_boot.trn_boot import _ntff_profile_via_ctypes
        mod.set_axon_ntff_profile_hook(
            _ntff_profile_via_ctypes("/opt/axon/libaxon_pjrt.so")
        )
    except Exception:
        pass


def _rel_indices(ny, nx):
    y = np.arange(ny)
    x = np.arange(nx)
    y1, x1, y2, x2 = np.meshgrid(y, x, y, x, indexing="ij")
    idx = (y1 - y2 + ny - 1) * (2 * nx - 1) + (x1 - x2 + nx - 1)
    return idx.reshape(ny * nx, ny * nx)


def _build():
    """Build + bacc-compile the per-core program (cached)."""
    if os.environ.get("KLDW", "0") == "1":
        _patch_ldw_opt()
    stage = int(os.environ.get("KSTAGE", "4"))
    key = ("nc", stage, os.environ.get("KSUB", "4"))
    if key in _CACHE:
        return _CACHE[key]

    nc = bacc.Bacc("TRN2", target_bir_lowering=False, debug=False,
                   num_devices=NCORES)

    d_x2 = nc.dram_tensor("x2", [BPC, E, N], F16, kind="ExternalInput")
    d_wall = nc.dram_tensor("wall", [E, 4, E], F16, kind="ExternalInput")
    d_bq = nc.dram_tensor("bqv", [E, 1], F32, kind="ExternalInput")
    d_bf = nc.dram_tensor("bfv", [E, 1], F32, kind="ExternalInput")
    d_rel = nc.dram_tensor("relb", [H, NT, 128, N], F16, kind="ExternalInput")
    d_id = nc.dram_tensor("ident", [128, 128], F16, kind="ExternalInput")
    d_sel = nc.dram_tensor("sel4", [128, E], F16, kind="ExternalInput")
    d_y2 = nc.dram_tensor("y2", [BPC, E, N], F32, kind="ExternalOutput")

    from concourse.tile_rust import add_dep_helper

    def noldw(mm):
        (mm.ins if hasattr(mm, "ins") else mm).ldweights = False

    def raw(mm):
        return mm.ins if hasattr(mm, "ins") else mm

    def order(a, b):
        add_dep_helper(raw(a), raw(b), sync=False,
                       reason="pin PE order for weight reuse")

    with nc.allow_low_precision(reason="fp32r matmul operand tiles"), \
         tile.TileContext(nc) as tc:
        with (
            tc.tile_pool(name="const", bufs=1) as const,
            tc.tile_pool(name="persist", bufs=1) as persist,
            tc.tile_pool(name="xp", bufs=1) as xp,
            tc.tile_pool(name="relp", bufs=4) as relp,
            tc.tile_pool(name="Ep", bufs=4) as Ep,
            tc.tile_pool(name="yp", bufs=1) as yp,
            tc.tile_pool(name="ps", bufs=3, space="PSUM") as ps,
            tc.tile_pool(name="pso", bufs=1, space="PSUM") as pso,
        ):
            wall_sb = const.tile([E, 4, E], F16, tag="wall")
            wk_sb = wall_sb[:, 0]
            wq_sb = wall_sb[:, 1]
            wv_sb = wall_sb[:, 2]
            wu_sb = wall_sb[:, 3]
            id_sb = const.tile([128, 128], F16, tag="id")
            sel_sb = const.tile([128, E], F16, tag="sel")
            bq_sb = const.tile([E, 1], F32, tag="bq")
            bf_sb = const.tile([E, 1], F32, tag="bf")
            nc.sync.dma_start(wall_sb[:], d_wall.ap()[:])
            nc.sync.dma_start(id_sb[:], d_id.ap()[:])
            nc.sync.dma_start(sel_sb[:], d_sel.ap()[:])
            nc.sync.dma_start(bq_sb[:], d_bq.ap()[:])
            nc.sync.dma_start(bf_sb[:], d_bf.ap()[:])

            K_sb, Q_sb, VT_sb, out_sb, R_sb, R32_sb, RD_sb = {}, {}, {}, {}, {}, {}, {}
            x_tiles = {}
            for b in range(BPC):
                x_tiles[b] = xp.tile([E, N], F16, tag=f"x{b}", name=f"x{b}")
                nc.sync.dma_start(x_tiles[b][:], d_x2.ap()[b])
            rel_t = {}
            dma_engines = [nc.sync]
            for h in range(H):
                rel_t[h] = relp.tile([128, NT, N], F16, tag="rel",
                                     name=f"rel{h}")
                nc.sync.dma_start(rel_t[h][:],
                                  d_rel.ap()[h].rearrange("t p q -> p t q"))
            for b in range(BPC):
                K_sb[b] = persist.tile([E, N], F16, tag=f"K{b}", name=f"K{b}")
                Q_sb[b] = persist.tile([E, N], F16, tag=f"Q{b}", name=f"Q{b}")
                VT_sb[b] = persist.tile([128, NT, H, HC + 1], F16, tag=f"VT{b}", name=f"VT{b}")
                out_sb[b] = persist.tile([E, N], F16, tag=f"O{b}", name=f"O{b}")
                R_sb[b] = persist.tile([128, N], F16, tag=f"R{b}", name=f"R{b}")
                R32_sb[b] = persist.tile([128, N], F32, tag=f"R32{b}", name=f"R32{b}")
                RD_sb[b] = persist.tile([128, N], F32, tag=f"RD{b}", name=f"RD{b}")

            # ---- projections ----
            for b in range(BPC):
                x_sb = x_tiles[b]
                nc.vector.memset(VT_sb[b][:], 1.0)
                nc.vector.memset(R_sb[b][:], 0.0)
                nc.vector.memset(RD_sb[b][:], 1.0)
                pks, pqs = [], []
                for j in range(2):
                    js = ds(512 * j, 512)
                    pk = ps.tile([128, 512], F32, tag="ps", name=f"pk{j}")
                    mm = nc.tensor.matmul(pk[:], wk_sb, x_sb[:, js],
                                          start=True, stop=True)
                    if j == 1:
                        noldw(mm)
                    pks.append(pk)
                for j in range(2):
                    js = ds(512 * j, 512)
                    pq = ps.tile([128, 512], F32, tag="ps", name=f"pq{j}")
                    mm = nc.tensor.matmul(pq[:], wq_sb, x_sb[:, js],
                                          start=True, stop=True)
                    if j == 1:
                        noldw(mm)
                    pqs.append(pq)
                for j in range(2):
                    js = ds(512 * j, 512)
                    nc.scalar.copy(K_sb[b][:, js], pks[j][:])
                    nc.vector.tensor_scalar_add(Q_sb[b][:, js], pqs[j][:], bq_sb[:])
                for t in range(NT):
                    pv = ps.tile([128, 128], F32, tag="ps")
                    nc.tensor.matmul(pv[:], x_sb[:, ts(t, 128)], wv_sb,
                                     start=True, stop=True)
                    nc.vector.tensor_copy(
                        VT_sb[b][:, t, :, 0:HC],
                        pv[:].rearrange("p (h c) -> p h c", h=H),
                    )

            # ---- divide + output projection (emitted per-sample after its
            # last head pair so it overlaps the other sample's attention) ----
            def emit_divide(b):
                nc.vector.reciprocal_approx_fast(out=R32_sb[b][:], in_=RD_sb[b][:])
                nc.vector.tensor_copy(R_sb[b][:], R32_sb[b][:])
                pbc = pso.tile([128, N], F32, tag="pso", name=f"pbc{b}")
                for j in range(2):
                    js = ds(512 * j, 512)
                    mm = nc.tensor.matmul(pbc[:, js], sel_sb[:], R_sb[b][:, js],
                                          start=True, stop=True)
                    if j == 1:
                        noldw(mm)
                nc.vector.tensor_mul(out_sb[b][:], out_sb[b][:], pbc[:])
                py = pso.tile([128, N], F32, tag="pso", name=f"py{b}")
                for j in range(2):
                    js = ds(512 * j, 512)
                    mm = nc.tensor.matmul(py[:, js], wu_sb, out_sb[b][:, js],
                                          start=True, stop=True)
                    if j == 1:
                        noldw(mm)
                y_sb = yp.tile([E, N], F32, tag="y", name=f"ysb{b}")
                nc.vector.tensor_scalar_add(y_sb[:], py[:], bf_sb[:])
                nc.sync.dma_start(d_y2.ap()[b], y_sb[:])

            # ---- attention, head pairs ----
            for p in range(2 if stage >= 2 else 0):
                hs = (2 * p, 2 * p + 1)
                for b in range(BPC):
                    Et = {h: Ep.tile([128, NT, N], F16, tag="E", name=f"E{h}") for h in hs}
                    for t in range(NT):
                        pa = {}
                        for h in hs:
                            pa[h] = ps.tile([128, N], F32, tag="ps",
                                            name=f"pa{h}")
                        for h in hs:
                            for j in range(2):
                                mm = nc.tensor.matmul(
                                    pa[h][:, ds(512 * j, 512)], id_sb[:],
                                    rel_t[h][:, t, ds(512 * j, 512)],
                                    start=True, stop=False,
                                )
                                if j == 1:
                                    noldw(mm)
                        for h in hs:
                            for j in range(2):
                                mm = nc.tensor.matmul(
                                    pa[h][:, ds(512 * j, 512)],
                                    K_sb[b][ds(HC * h, HC), ts(t, 128)],
                                    Q_sb[b][ds(HC * h, HC), ds(512 * j, 512)],
                                    start=False, stop=True,
                                    tile_position=(HC * h, 0),
                                )
                                if j == 1:
                                    noldw(mm)
                        for h in hs:
                            nc.scalar.activation(Et[h][:, t, :], pa[h][:], AF.Exp)
                    # second matmul: out_h^num / D, heads separately
                    for h in (hs if stage >= 3 else ()):
                        po = pso.tile([HC + 1, N], F32, tag="pso")
                        for t in range(NT):
                            for j in range(2):
                                mm = nc.tensor.matmul(
                                    po[:, ds(512 * j, 512)],
                                    VT_sb[b][:, t, h, :],
                                    Et[h][:, t, ds(512 * j, 512)],
                                    start=(t == 0), stop=(t == NT - 1),
                                )
                                if j == 1:
                                    noldw(mm)
                        nc.vector.tensor_copy(RD_sb[b][ds(32 * h, 1), :], po[HC:HC + 1, :])
                        if p == 1 and b == BPC - 1:
                            nc.scalar.copy(out_sb[b][ds(HC * h, HC), :],
                                           po[0:HC, :])
                        else:
                            nc.vector.tensor_copy(out_sb[b][ds(HC * h, HC), :],
                                                  po[0:HC, :])
                    if p == 1 and stage >= 4:
                        emit_divide(b)



            if stage < 4 or int(os.environ.get("KSUB", "4")) < 3:
                for b in range(BPC):
                    nc.gpsimd.dma_start(d_y2.ap()[b], K_sb[b][:])

    nc.compile()
    _CACHE[key] = nc
    return nc


def kernel(x, Wk, bk, Wq, bq, Wv, bv, Wu, bu, pos_enc):
    global LAST_RESULT
    x = np.ascontiguousarray(np.asarray(x, np.float32))
    Wk = np.asarray(Wk, np.float32)
    Wq = np.asarray(Wq, np.float32)
    Wv = np.asarray(Wv, np.float32)
    Wu = np.asarray(Wu, np.float32)
    bq = np.asarray(bq, np.float32)
    bv = np.asarray(bv, np.float32)
    bu = np.asarray(bu, np.float32)
    pos_enc = np.asarray(pos_enc, np.float32)

    wall = np.stack([Wk.T, (Wq * SCALE).T, Wv.T, Wu.T], axis=1)
    wall = np.ascontiguousarray(wall.astype(np.float16))
    bqv = np.ascontiguousarray((bq * SCALE).reshape(E, 1))
    bfv = np.ascontiguousarray((Wu @ bv + bu).reshape(E, 1))

    idx = _rel_indices(NY, NX)
    rel = pos_enc[:, idx]                         # (H, N, N) fp32
    relb = np.ascontiguousarray(
        rel.reshape(H, NT, 128, N).astype(np.float16))
    ident = np.eye(128, dtype=np.float16)
    sel4 = np.zeros((128, E), np.float16)
    for h in range(H):
        sel4[32 * h, HC * h:HC * (h + 1)] = 1.0

    nc = _build()

    common = dict(wall=wall, bqv=bqv, bfv=bfv,
                  relb=relb, ident=ident, sel4=sel4)
    in_maps = []
    xr = x.reshape(B, E, N)
    for c in range(NCORES):
        m = dict(common)
        m["x2"] = np.ascontiguousarray(xr[BPC * c:BPC * (c + 1)].astype(np.float16))
        in_maps.append(m)

    trace = os.environ.get("BASS_TRACE", "") not in ("", "0")
    if trace:
        _ensure_ntff_hook()
    res = bass_utils.run_bass_kernel_spmd(
        nc, in_maps, core_ids=list(range(NCORES)), trace=trace)
    LAST_RESULT = res

    y = np.empty((B, E, N), np.float32)
    for c in range(NCORES):
        y[BPC * c:BPC * (c + 1)] = res.results[c]["y2"]
    return y.reshape(B, E, NY, NX)



# revision 15
# speedup vs baseline: 1.2813x; 1.2813x over previous
"""Attention2d Trainium2 kernel.

Reference computation (per sample b):
  K = Wk @ x;  Q = Wq @ x + bq;  V = Wv @ x + bv     (x: [128, 1024])
  per head h (32 channels):  att[k,q] = scale * K_h[:,k].Q_h[:,q] + rel_h[k,q]
  P = softmax_k(att);  out_h = V_h @ P;  y = Wu @ out + bu

Kernel strategy (8 NeuronCores, data-parallel over batch, 2 samples/core):
  - rel_h is a fixed positional-bias matrix.  Per 128-row k-tile we
    column-center it (the per-q mean is softmax-invariant and dropped) and
    factor the rest with a rank-96 SVD: rel_tile ~= A^T B.  The factors ride
    in the 96 unused contraction rows of the K.Q matmul (head channels use
    only 32 of 128), so the rel add costs zero extra engine time anywhere.
    Measured end-to-end rel-err of this approximation: ~3.3e-3.
  - score matmul per (h, k-tile): stationary [128,128] = K_h rows at
    partitions 32h..32h+32 plus permuted A rows, moving [128, 1024] = Q_h
    (replicated per k-tile by SBUF->SBUF DMA) plus B rows.
  - exp on ScalarE (the critical path, ~1.0us per 128x1024 tile).
  - softmax denominator D[q] via an appended ones-column in the V^T
    stationary operand of the second matmul; division applied after it via
    a selector-matmul partition-broadcast of 1/D.
  - ~20 dummy matmuls at kernel start keep the PE busy during the DMA
    prefetch so the HAM clock-gate reaches 2.4 GHz before real work.
"""

import os
import sys
import types
import hashlib

sys.path.insert(0, "/opt/trn_rl_repo")

import numpy as np

import concourse.bass as bass
import concourse.tile as tile
from concourse import bacc, mybir
from concourse import bass_utils
from concourse.bass import ds, ts

F32 = mybir.dt.float32
F16 = mybir.dt.float16
AF = mybir.ActivationFunctionType

B, E, H, NY, NX = 16, 128, 4, 32, 32
N = NY * NX          # 1024
HC = E // H          # 32
NCORES = 8
BPC = B // NCORES    # 2 samples per core
NT = N // 128        # 8 k-tiles
RK = 96              # rank of the per-tile rel factorization
SCALE = HC ** -0.5
NDUMMY = 20          # PE warm-up matmuls

LAST_RESULT = None   # BassKernelResults of the most recent run (for test.py)

_CACHE = {}


def _ensure_ntff_hook():
    """Register the axon NTFF profile hook that trn_boot couldn't install
    (the image lacks antenv.axon_hooks). Only needed when tracing."""
    if "antenv.axon_hooks" in sys.modules:
        return
    mod = types.ModuleType("antenv.axon_hooks")
    holder = [None]
    mod.set_axon_ntff_profile_hook = lambda h: holder.__setitem__(0, h)
    mod.get_axon_ntff_profile_hook = lambda: holder[0]
    sys.modules["antenv.axon_hooks"] = mod
    try:
        from trn_agent_boot.trn_boot import _ntff_profile_via_ctypes
        mod.set_axon_ntff_profile_hook(
            _ntff_profile_via_ctypes("/opt/axon/libaxon_pjrt.so")
        )
    except Exception:
        pass


def _rel_indices(ny, nx):
    y = np.arange(ny)
    x = np.arange(nx)
    y1, x1, y2, x2 = np.meshgrid(y, x, y, x, indexing="ij")
    idx = (y1 - y2 + ny - 1) * (2 * nx - 1) + (x1 - x2 + nx - 1)
    return idx.reshape(ny * nx, ny * nx)


def _rel_factors(pos_enc):
    """Per-(h, k-tile) rank-RK factors of the column-centered rel matrix,
    already laid out for SBUF: K rows (partitions 32h..32h+32) zeroed, the
    96 A/B rows permuted into the complementary partitions."""
    key = ("fac", hashlib.sha1(pos_enc.tobytes()).hexdigest())
    if key in _CACHE:
        return _CACHE[key]
    idx = _rel_indices(NY, NX)
    rel = pos_enc[:, idx]                         # (H, N, N) fp32
    ka0 = np.zeros((H, 128, N), np.float16)
    qb0 = np.zeros((H, NT, 128, N), np.float16)
    for h in range(H):
        comp = np.array([p for p in range(128)
                         if not (HC * h <= p < HC * (h + 1))])
        for t in range(NT):
            M = rel[h, 128 * t:128 * (t + 1), :].astype(np.float32)
            M = M - M.mean(axis=0, keepdims=True)  # per-q shift: softmax-inv
            U, S, Vt = np.linalg.svd(M, full_matrices=False)
            rs = np.sqrt(S[:RK])
            ka0[h][comp, 128 * t:128 * (t + 1)] = (U[:, :RK] * rs).T
            qb0[h, t][comp, :] = rs[:, None] * Vt[:RK]
    _CACHE[key] = (ka0, qb0)
    return ka0, qb0


def _build():
    """Build + bacc-compile the per-core program (cached)."""
    if "nc" in _CACHE:
        return _CACHE["nc"]

    nc = bacc.Bacc("TRN2", target_bir_lowering=False, debug=False,
                   num_devices=NCORES)

    d_x2 = nc.dram_tensor("x2", [BPC, E, N], F16, kind="ExternalInput")
    d_wall = nc.dram_tensor("wall", [E, 4, E], F16, kind="ExternalInput")
    d_bq = nc.dram_tensor("bqv", [E, 1], F32, kind="ExternalInput")
    d_bf = nc.dram_tensor("bfv", [E, 1], F32, kind="ExternalInput")
    d_ka = nc.dram_tensor("ka0", [H, 128, N], F16, kind="ExternalInput")
    d_qb = nc.dram_tensor("qb0", [H, NT, 128, N], F16, kind="ExternalInput")
    d_sel = nc.dram_tensor("sel4", [128, E], F16, kind="ExternalInput")
    d_y2 = nc.dram_tensor("y2", [BPC, E, N], F32, kind="ExternalOutput")
    dbg = os.environ.get("KDBG", "0") == "1"
    if dbg:
        d_dbgq = nc.dram_tensor("dbgq", [128, NT, N], F16,
                                kind="ExternalOutput")
        d_dbgk = nc.dram_tensor("dbgk", [128, N], F16, kind="ExternalOutput")
        d_dbgs = nc.dram_tensor("dbgs", [E, N], F16, kind="ExternalOutput")

    def noldw(mm):
        (mm.ins if hasattr(mm, "ins") else mm).ldweights = False

    with nc.allow_low_precision(reason="fp16 matmul operand tiles"), \
         tile.TileContext(nc) as tc:
        with (
            tc.tile_pool(name="const", bufs=1) as const,
            tc.tile_pool(name="kap", bufs=1) as kap,
            tc.tile_pool(name="qbp", bufs=1) as qbp,
            tc.tile_pool(name="persist", bufs=1) as persist,
            tc.tile_pool(name="Ep", bufs=4) as Ep,
            tc.tile_pool(name="yp", bufs=1) as yp,
            tc.tile_pool(name="ps", bufs=2, space="PSUM") as ps,
            tc.tile_pool(name="pso", bufs=2, space="PSUM") as pso,
        ):
            wall_sb = const.tile([E, 4, E], F16, tag="wall")
            wk_sb = wall_sb[:, 0]
            wq_sb = wall_sb[:, 1]
            wv_sb = wall_sb[:, 2]
            wu_sb = wall_sb[:, 3]
            sel_sb = const.tile([128, E], F16, tag="sel")
            bq_sb = const.tile([E, 1], F32, tag="bq")
            bf_sb = const.tile([E, 1], F32, tag="bf")
            scr_sb = const.tile([128, 128], F16, tag="scr")

            # ---- PE warm-up: dummy matmuls with no DMA dependencies ----
            nc.gpsimd.memset(scr_sb[:], 0.0)
            dum = pso.tile([128, 128], F32, tag="pso", name="dummy")
            for i in range(NDUMMY):
                mm = nc.tensor.matmul(dum[:], scr_sb[:], scr_sb[:],
                                      start=True, stop=True)
                if i:
                    noldw(mm)

            # ---- DMA prefetch, spread across idle engine queues ----
            x_tiles = {}
            for b in range(BPC):
                x_tiles[b] = persist.tile([E, N], F16, tag=f"x{b}", name=f"x{b}")
            nc.sync.dma_start(x_tiles[0][:], d_x2.ap()[0])
            nc.sync.dma_start(wall_sb[:], d_wall.ap()[:])
            nc.sync.dma_start(x_tiles[1][:], d_x2.ap()[1])
            nc.sync.dma_start(sel_sb[:], d_sel.ap()[:])
            nc.scalar.dma_start(bq_sb[:], d_bq.ap()[:])
            nc.scalar.dma_start(bf_sb[:], d_bf.ap()[:])

            KA, QB = {}, {}
            for h in range(H):
                KA[h] = kap.tile([128, N], F16, tag=f"KA{h}", name=f"KA{h}")
                QB[h] = qbp.tile([128, NT, N], F16, tag=f"QB{h}", name=f"QB{h}")
            for h in range(H):
                nc.scalar.dma_start(KA[h][:], d_ka.ap()[h])
            # head 0 split so its first k-tiles land early
            nc.gpsimd.dma_start(
                QB[0][:, 0:4, :],
                d_qb.ap()[0, 0:4].rearrange("t p q -> p t q"))
            nc.gpsimd.dma_start(
                QB[0][:, 4:NT, :],
                d_qb.ap()[0, 4:NT].rearrange("t p q -> p t q"))
            for h in range(1, H):
                nc.gpsimd.dma_start(
                    QB[h][:], d_qb.ap()[h].rearrange("t p q -> p t q"))

            K_sb, Q_sb, VT_sb, out_sb = {}, {}, {}, {}
            R_sb, R32_sb, RD_sb = {}, {}, {}
            for b in range(BPC):
                K_sb[b] = persist.tile([E, N], F16, tag=f"K{b}", name=f"K{b}")
                Q_sb[b] = persist.tile([E, N], F16, tag=f"Q{b}", name=f"Q{b}")
                VT_sb[b] = persist.tile([128, NT, H, HC + 1], F16,
                                        tag=f"VT{b}", name=f"VT{b}")
                out_sb[b] = persist.tile([E, N], F16, tag=f"O{b}", name=f"O{b}")
                R_sb[b] = persist.tile([128, N], F16, tag=f"R{b}", name=f"R{b}")
                R32_sb[b] = persist.tile([128, N], F32, tag=f"R32{b}", name=f"R32{b}")
                RD_sb[b] = persist.tile([128, N], F32, tag=f"RD{b}", name=f"RD{b}")
            for b in range(BPC):
                nc.vector.memset(VT_sb[b][:], 1.0)
                nc.vector.memset(RD_sb[b][:], 1.0)

            # ---- projections (both samples up front) ----
            for b in range(BPC):
                x_sb = x_tiles[b]
                pk = ps.tile([128, N], F32, tag="ps", name=f"pk{b}")
                for j in range(2):
                    js = ds(512 * j, 512)
                    mm = nc.tensor.matmul(pk[:, js], wk_sb, x_sb[:, js],
                                          start=True, stop=True)
                    if j == 1:
                        noldw(mm)
                nc.vector.tensor_copy(K_sb[b][:], pk[:])
                if b == 0:
                    for h in range(H):
                        hs = ds(HC * h, HC)
                        nc.sync.dma_start(KA[h][hs, :], K_sb[b][hs, :])
                pq = ps.tile([128, N], F32, tag="ps", name=f"pq{b}")
                for j in range(2):
                    js = ds(512 * j, 512)
                    mm = nc.tensor.matmul(pq[:, js], wq_sb, x_sb[:, js],
                                          start=True, stop=True)
                    if j == 1:
                        noldw(mm)
                nc.vector.tensor_scalar_add(Q_sb[b][:], pq[:], bq_sb[:])
                if b == 0:
                    for h in range(H):
                        # replicate Q_h into the 8 k-tile slots of QB[h];
                        # independent DMAs (a chained slot-doubling races:
                        # DMAs on one queue overlap in flight)
                        hs = ds(HC * h, HC)
                        for t in range(NT):
                            nc.sync.dma_start(QB[h][hs, t, :],
                                              Q_sb[b][hs, :])
                if dbg and b == 0:
                    nc.scalar.dma_start(d_dbgq.ap()[:], QB[0][:])
                    nc.scalar.dma_start(d_dbgk.ap()[:], KA[0][:])
                    nc.scalar.dma_start(d_dbgs.ap()[:], Q_sb[0][:])
                pv = ps.tile([128, NT, 128], F32, tag="ps", name=f"pv{b}")
                for t in range(NT):
                    nc.tensor.matmul(pv[:, t, :], x_sb[:, ts(t, 128)], wv_sb,
                                     start=True, stop=True)
                nc.vector.tensor_copy(
                    VT_sb[b][:, :, :, 0:HC],
                    pv[:].rearrange("p t (h c) -> p t h c", h=H),
                )

            # ---- divide + output projection ----
            def emit_divide(b):
                nc.vector.reciprocal_approx_fast(out=R32_sb[b][:],
                                                 in_=RD_sb[b][:])
                nc.vector.tensor_copy(R_sb[b][:], R32_sb[b][:])
                pbc = pso.tile([128, N], F32, tag="pso", name=f"pbc{b}")
                for j in range(2):
                    js = ds(512 * j, 512)
                    mm = nc.tensor.matmul(pbc[:, js], sel_sb[:],
                                          R_sb[b][:, js],
                                          start=True, stop=True)
                    if j == 1:
                        noldw(mm)
                nc.vector.tensor_mul(out_sb[b][:], out_sb[b][:], pbc[:])
                py = pso.tile([128, N], F32, tag="pso", name=f"py{b}")
                for j in range(2):
                    js = ds(512 * j, 512)
                    mm = nc.tensor.matmul(py[:, js], wu_sb,
                                          out_sb[b][:, js],
                                          start=True, stop=True)
                    if j == 1:
                        noldw(mm)
                y_sb = yp.tile([E, N], F32, tag="y", name=f"ysb{b}")
                nc.vector.tensor_scalar_add(y_sb[:], py[:], bf_sb[:])
                nc.sync.dma_start(d_y2.ap()[b], y_sb[:])

            # ---- score -> exp -> apply pipeline ----
            Et = {}
            po = {}

            def emit_apply(b, h, t):
                if t == 0:
                    po[(b, h)] = pso.tile([HC + 1, N], F32, tag="pso",
                                          name=f"po{b}{h}")
                p = po[(b, h)]
                for j in range(2):
                    js = ds(512 * j, 512)
                    mm = nc.tensor.matmul(p[:, js], VT_sb[b][:, t, h, :],
                                          Et[(b, h)][:, t, js],
                                          start=(t == 0), stop=(t == NT - 1))
                    if j == 1:
                        noldw(mm)
                if t == NT - 1:
                    nc.vector.tensor_copy(RD_sb[b][ds(HC * h, 1), :],
                                          p[HC:HC + 1, :])
                    nc.vector.tensor_copy(out_sb[b][ds(HC * h, HC), :],
                                          p[0:HC, :])
                    if h == H - 1:
                        emit_divide(b)

            units = [(b, h, t) for b in range(BPC) for h in range(H)
                     for t in range(NT)]
            pending = []
            for b, h, t in units:
                if t == 0:
                    Et[(b, h)] = Ep.tile([128, NT, N], F16, tag="E",
                                         name=f"E{b}{h}")
                pa = ps.tile([128, N], F32, tag="ps", name=f"pa{b}{h}{t}")
                for j in range(2):
                    js = ds(512 * j, 512)
                    mm = nc.tensor.matmul(pa[:, js], KA[h][:, ts(t, 128)],
                                          QB[h][:, t, js],
                                          start=True, stop=True)
                    if j == 1:
                        noldw(mm)
                nc.scalar.activation(Et[(b, h)][:, t, :], pa[:], AF.Exp)
                if b == 0 and t == NT - 1:
                    # sample 1's K/Q go into the shared KA/QB tiles only
                    # after sample 0's scores for this head consumed them
                    hs = ds(HC * h, HC)
                    nc.gpsimd.dma_start(KA[h][hs, :], K_sb[1][hs, :])
                    for tt in range(NT):
                        nc.gpsimd.dma_start(QB[h][hs, tt, :], Q_sb[1][hs, :])
                pending.append((b, h, t))
                if len(pending) > 2:
                    emit_apply(*pending.pop(0))
            for u in pending:
                emit_apply(*u)

    nc.compile()
    _CACHE["nc"] = nc
    return nc


def kernel(x, Wk, bk, Wq, bq, Wv, bv, Wu, bu, pos_enc):
    global LAST_RESULT
    x = np.ascontiguousarray(np.asarray(x, np.float32))
    Wk = np.asarray(Wk, np.float32)
    Wq = np.asarray(Wq, np.float32)
    Wv = np.asarray(Wv, np.float32)
    Wu = np.asarray(Wu, np.float32)
    bq = np.asarray(bq, np.float32)
    bv = np.asarray(bv, np.float32)
    bu = np.asarray(bu, np.float32)
    pos_enc = np.asarray(pos_enc, np.float32)

    wall = np.stack([Wk.T, (Wq * SCALE).T, Wv.T, Wu.T], axis=1)
    wall = np.ascontiguousarray(wall.astype(np.float16))
    bqv = np.ascontiguousarray((bq * SCALE).reshape(E, 1))
    bfv = np.ascontiguousarray((Wu @ bv + bu).reshape(E, 1))

    ka0, qb0 = _rel_factors(pos_enc)
    sel4 = np.zeros((128, E), np.float16)
    for h in range(H):
        sel4[32 * h, HC * h:HC * (h + 1)] = 1.0

    nc = _build()

    common = dict(wall=wall, bqv=bqv, bfv=bfv,
                  ka0=ka0, qb0=qb0, sel4=sel4)
    in_maps = []
    xr = x.reshape(B, E, N)
    for c in range(NCORES):
        m = dict(common)
        m["x2"] = np.ascontiguousarray(
            xr[BPC * c:BPC * (c + 1)].astype(np.float16))
        in_maps.append(m)

    trace = os.environ.get("BASS_TRACE", "") not in ("", "0")
    if trace:
        _ensure_ntff_hook()
    res = bass_utils.run_bass_kernel_spmd(
        nc, in_maps, core_ids=list(range(NCORES)), trace=trace)
    LAST_RESULT = res

    y = np.empty((B, E, N), np.float32)
    for c in range(NCORES):
        y[BPC * c:BPC * (c + 1)] = res.results[c]["y2"]
    return y.reshape(B, E, NY, NX)
